# revision 1
# baseline (speedup 1.0000x reference)
"""MoE fused top-k-gating decode kernel for Trainium2 (8 NeuronCores).

Problem: B=32,S=1,H=2048, E=8 experts, I=5632, top_k=2, fp32.
Sharding: expert-parallel - core c owns expert c (w_gate/w_up/w_down[c]),
router weight replicated (rolled per-core so column 0 is the own expert).
Each core computes the full router (softmax + top-2 mask), its expert's
GLU-MLP for all 32 tokens, scales by its combine weight, and returns a
partial [T,H] output; the host sums the 8 partials.

Per-core roofline: 3*H*I*4B = 138.4 MB of weight DMA @ ~360 GB/s = ~390 us.
Matmuls keep the 32-token activations stationary ([128,32] lhsT tiles) and
stream the weights as the moving operand in float32r (1 cycle/row at N>=512;
e8m11 'TF32-like' rounding, weights pre-rounded RNE on host).

TRN2 allows only ONE sync-wait per instruction; the Bacc layer splits
excess waits into event-semaphore chains. The kernel is still arranged so
most PE instructions need at most one new semaphore (operand pairs arrive
in a single DMA, PSUM is read only by the vector engine, junk transposes
absorb DMA ticks), keeping the inserted event chains off the hot path.
"""

import numpy as np

import concourse.bass as bass
import concourse.bacc as bacc
import concourse.mybir as mybir
import concourse.tile as tile
from concourse.masks import make_identity
from concourse.bass_utils import run_bass_kernel_spmd

B, S, H = 32, 1, 2048
E, I = 8, 5632
T = B * S          # 32 tokens
P = 128            # partitions
NCORES = 8
SWIGLU_SCALE = 1.702

KH = H // P        # 16 contraction chunks over H
KI = I // P        # 44 contraction chunks over I
NW = 512           # moving-dim tile width
NT = I // NW       # 11 gate/up column slabs
ND = H // NW       # 4 down output tiles
XW = T + E         # packed xT+router width (40)

F32 = mybir.dt.float32
F32R = mybir.dt.float32r
AX = mybir.AxisListType.X
AF = mybir.ActivationFunctionType
OP = mybir.AluOpType


def _build_nc(use_f32r: bool = True) -> bass.Bass:
    nc = bacc.Bacc()

    WDT = F32R if use_f32r else F32
    xrw_d = nc.declare_dram_parameter("xrw", [H, XW], WDT, isOutput=False)
    wg_d = nc.declare_dram_parameter("wg", [H, I], WDT, isOutput=False)
    wu_d = nc.declare_dram_parameter("wu", [H, I], WDT, isOutput=False)
    wd_d = nc.declare_dram_parameter("wd", [I, H], WDT, isOutput=False)
    out_d = nc.declare_dram_parameter("out", [T, H], F32, isOutput=True)

    def asF32(ap):
        return ap.bitcast(F32) if use_f32r else ap

    with tile.TileContext(nc) as tc:
        with tc.tile_pool(name="const", bufs=1) as const:
            id_sb = const.tile([T, T], F32, name="id_sb")
            make_identity(nc, id_sb)

            xrw_sb = const.tile([P, KH * XW], WDT, name="xrw_sb")
            nc.sync.dma_start(
                out=xrw_sb.rearrange("p (k c) -> p k c", c=XW),
                in_=xrw_d.rearrange("(k p) c -> p k c", p=P),
            )

            interT_sb = const.tile([P, KI * T], WDT, name="interT_sb")
            out_sb = const.tile([T, H], F32, name="out_sb")
            comb_sb = const.tile([T, 1], F32, name="comb_sb")

            def xT_k(k):  # [128, 32] stationary activation chunk
                return xrw_sb[:, k * XW : k * XW + T]

            def rw_k(k):  # [128, 8] router weight chunk
                return xrw_sb[:, k * XW + T : (k + 1) * XW]

            widths = [NW] * (NT - 1) + [NW // 2, NW // 2]
            assert sum(widths) == I
            wg_cols = wg_d.rearrange("(k p) i -> p k i", p=P)
            wu_cols = wu_d.rearrange("(k p) i -> p k i", p=P)
            wgp = tc.alloc_tile_pool(name="wgp", bufs=2)
            wup = tc.alloc_tile_pool(name="wup", bufs=2)
            wdp = tc.alloc_tile_pool(name="wdp", bufs=3)

            # ---------------- router: softmax + top-2 mask ----------------
            with (
                tc.tile_pool(name="rps", bufs=1, space="PSUM") as rps,
                tc.tile_pool(name="rsb", bufs=1) as rsb,
            ):
                # absorb the ident DMA tick on PE before anything else
                dmy_ps = rps.tile([T, T], F32, name="dmy_ps", tag="dmy")
                nc.tensor.transpose(dmy_ps, id_sb, id_sb)

                logits = rps.tile([T, E], F32, name="logits", tag="logits")
                for k in range(KH):
                    nc.tensor.matmul(
                        logits,
                        asF32(xT_k(k)),
                        asF32(rw_k(k)),
                        start=(k == 0),
                        stop=(k == KH - 1),
                    )
                # PSUM is read only by DVE (keeps later PE writers 1-wait)
                lg = rsb.tile([T, E], F32, name="lg")
                nc.vector.tensor_copy(lg, logits)
                mx = rsb.tile([T, 1], F32, name="mx")
                nc.vector.reduce_max(mx, lg, axis=AX)
                nmx = rsb.tile([T, 1], F32, name="nmx")
                nc.vector.tensor_scalar_mul(nmx, mx, -1.0)
                ex = rsb.tile([T, E], F32, name="ex")
                nc.scalar.activation(ex, lg, AF.Exp, bias=nmx, scale=1.0)
                sm = rsb.tile([T, 1], F32, name="sm")
                nc.vector.reduce_sum(sm, ex, axis=AX)
                rc = rsb.tile([T, 1], F32, name="rc")
                nc.vector.reciprocal(rc, sm)
                aff = rsb.tile([T, E], F32, name="aff")
                nc.vector.tensor_scalar_mul(aff, ex, rc)
                # top-2: value >= (second largest)
                m1 = rsb.tile([T, 1], F32, name="m1")
                nc.vector.reduce_max(m1, aff, axis=AX)
                pen = rsb.tile([T, E], F32, name="pen")
                nc.vector.tensor_scalar(
                    pen, aff, m1, -1e30, op0=OP.is_equal, op1=OP.mult
                )
                b2 = rsb.tile([T, E], F32, name="b2")
                nc.vector.tensor_add(b2, aff, pen)
                m2 = rsb.tile([T, 1], F32, name="m2")
                nc.vector.reduce_max(m2, b2, axis=AX)
                ge = rsb.tile([T, E], F32, name="ge")
                nc.vector.tensor_scalar(ge, aff, m2, None, op0=OP.is_ge)
                msk = rsb.tile([T, E], F32, name="msk")
                nc.vector.tensor_mul(msk, aff, ge)
                # rolled router weight puts the own expert at column 0
                nc.vector.tensor_copy(comb_sb, msk[:, 0:1])

            # ---- fused gate/up + swiglu + transpose + interleaved down ----
            # Down-matmul chunk ki only needs interT chunk ki (ready right
            # after slab ki//4's epilogue), so wd streams through the whole
            # kernel instead of serializing after gate/up. Slab widths taper
            # to 256 at the end to shrink the serial tail chain after the
            # last weight bytes land. PSUM: gate/up 2 + transpose 2 + down
            # accumulators 4 = 8 banks.
            with (
                tc.tile_pool(name="gup", bufs=1, space="PSUM") as gup,
                tc.tile_pool(name="tps", bufs=2, space="PSUM") as tps,
                tc.tile_pool(name="dps", bufs=1, space="PSUM") as dps,
                tc.tile_pool(name="esb", bufs=2) as esb,
            ):
                d_ps = [
                    dps.tile([T, NW], F32, name=f"d_ps{j}", tag=f"d{j}")
                    for j in range(ND)
                ]
                c0 = 0
                for n, w in enumerate(widths):
                    wg_sl = wgp.tile([P, KH * NW], WDT, name="wg_sl", tag="wg")
                    wu_sl = wup.tile([P, KH * NW], WDT, name="wu_sl", tag="wu")
                    if w == NW:
                        nc.sync.dma_start(
                            out=wg_sl.rearrange("p (k c) -> p k c", c=NW)[:, :, :w],
                            in_=wg_cols[:, :, c0 : c0 + w],
                        )
                        nc.sync.dma_start(
                            out=wu_sl.rearrange("p (k c) -> p k c", c=NW)[:, :, :w],
                            in_=wu_cols[:, :, c0 : c0 + w],
                        )
                    else:
                        # tail slabs: k-halves so matmuls overlap the stream
                        for kh in (slice(0, KH // 2), slice(KH // 2, KH)):
                            nc.sync.dma_start(
                                out=wg_sl.rearrange("p (k c) -> p k c", c=NW)[
                                    :, kh, :w
                                ],
                                in_=wg_cols[:, kh, c0 : c0 + w],
                            )
                        for kh in (slice(0, KH // 2), slice(KH // 2, KH)):
                            nc.sync.dma_start(
                                out=wu_sl.rearrange("p (k c) -> p k c", c=NW)[
                                    :, kh, :w
                                ],
                                in_=wu_cols[:, kh, c0 : c0 + w],
                            )
                    g_ps = gup.tile([T, NW], F32, name="g_ps", tag="g")
                    u_ps = gup.tile([T, NW], F32, name="u_ps", tag="u")
                    for k in range(KH):
                        nc.tensor.matmul(
                            g_ps[:, :w],
                            xT_k(k),
                            wg_sl[:, k * NW : k * NW + w],
                            start=(k == 0),
                            stop=(k == KH - 1),
                        )
                    for k in range(KH):
                        nc.tensor.matmul(
                            u_ps[:, :w],
                            xT_k(k),
                            wu_sl[:, k * NW : k * NW + w],
                            start=(k == 0),
                            stop=(k == KH - 1),
                        )
                    # epilogue: PSUM read only by DVE; sigmoid runs off a copy
                    g_sb = esb.tile([T, NW], F32, name="g_sb", tag="gsb")
                    nc.vector.tensor_copy(g_sb[:, :w], g_ps[:, :w])
                    sig = esb.tile([T, NW], F32, name="sig", tag="sig")
                    nc.scalar.activation(
                        sig[:, :w], g_sb[:, :w], AF.Sigmoid, scale=SWIGLU_SCALE
                    )
                    t1 = esb.tile([T, NW], F32, name="t1", tag="t1")
                    nc.vector.tensor_mul(t1[:, :w], g_ps[:, :w], sig[:, :w])
                    inter = esb.tile([T, NW], F32, name="inter", tag="inter")
                    nc.vector.tensor_mul(inter[:, :w], t1[:, :w], u_ps[:, :w])
                    for j in range(w // P):
                        ic = c0 // P + j
                        tp = tps.tile([P, T], F32, name="tp", tag="tp")
                        nc.tensor.transpose(tp, inter[:, j * P : (j + 1) * P], id_sb)
                        nc.vector.tensor_copy(
                            interT_sb[:, ic * T : (ic + 1) * T], tp
                        )
                    for ki in range(c0 // P, (c0 + w) // P):
                        wd_sl = wdp.tile([P, H], WDT, name="wd_sl", tag="wd")
                        nc.gpsimd.dma_start(
                            out=wd_sl, in_=wd_d[ki * P : (ki + 1) * P, :]
                        )
                        for j in range(ND):
                            nc.tensor.matmul(
                                d_ps[j],
                                interT_sb[:, ki * T : (ki + 1) * T],
                                wd_sl[:, j * NW : (j + 1) * NW],
                                start=(ki == 0),
                                stop=(ki == KI - 1),
                            )
                    c0 += w
                for j in range(ND):
                    nc.vector.tensor_scalar_mul(
                        out_sb[:, j * NW : (j + 1) * NW], d_ps[j], comb_sb
                    )
                    nc.sync.dma_start(
                        out=out_d[:, j * NW : (j + 1) * NW],
                        in_=out_sb[:, j * NW : (j + 1) * NW],
                    )
            wdp.release()
            wup.release()
            wgp.release()
    nc.finalize()
    return nc


def _rne_f32r(a: np.ndarray) -> np.ndarray:
    """Round fp32 to e8m11 (fp32r) with round-to-nearest-even on bit 12."""
    bits = a.view(np.uint32).astype(np.uint64)
    lsb = (bits >> 12) & 1
    bits = bits + 0x7FF + lsb
    bits &= 0xFFFFF000
    return bits.astype(np.uint32).view(np.float32)


def _make_in_maps(hidden_states, router_weight, w_gate, w_up, w_down,
                  round_weights=True):
    x = np.ascontiguousarray(np.asarray(hidden_states, np.float32).reshape(T, H))
    rw = np.asarray(router_weight, np.float32)
    wg = np.ascontiguousarray(np.asarray(w_gate, np.float32))
    wu = np.ascontiguousarray(np.asarray(w_up, np.float32))
    wd = np.ascontiguousarray(np.asarray(w_down, np.float32))
    if round_weights:
        wg = _rne_f32r(wg)
        wu = _rne_f32r(wu)
        wd = _rne_f32r(wd)
    xT = np.ascontiguousarray(x.T)  # [H, T], NOT rounded (router accuracy)
    in_maps = []
    for c in range(NCORES):
        order = [(j + c) % E for j in range(E)]  # column j holds expert (j+c)%E
        rwT = rw[order].T  # [H, E]; col 0 = own expert
        xrw = np.ascontiguousarray(np.concatenate([xT, rwT], axis=1))  # [H, T+E]
        in_maps.append(
            {
                "xrw": xrw,
                "wg": wg[c],
                "wu": wu[c],
                "wd": wd[c],
            }
        )
    return in_maps


def kernel(
    hidden_states,
    router_weight,
    w_gate,
    w_up,
    w_down,
    top_k,
    _trace: bool = False,
    _use_f32r: bool = True,
    _trace_all: bool = False,
):
    assert int(top_k) == 2, "kernel hardcodes top_k=2"
    in_maps = _make_in_maps(hidden_states, router_weight, w_gate, w_up, w_down,
                            round_weights=_use_f32r)
    nc = _build_nc(use_f32r=_use_f32r)
    res = run_bass_kernel_spmd(
        nc, in_maps, core_ids=list(range(NCORES)), trace=_trace,
        trace_cores=list(range(NCORES)) if (_trace and _trace_all) else None,
    )
    outs = np.stack([res.results[c]["out"] for c in range(NCORES)], axis=0)
    out = outs.sum(axis=0, dtype=np.float64).astype(np.float32)
    if _trace:
        kernel.last_exec_time_ns = res.exec_time_ns
        kernel.last_mean_exec_time_ns = res.mean_exec_time_ns
        kernel.last_trace = res.instructions_and_trace
    return out.reshape(B, S, H)



# revision 3
# speedup vs baseline: 1.8869x; 1.8869x over previous
"""MoE fused top-k-gating decode kernel for Trainium2 (8 NeuronCores).

Problem: B=32,S=1,H=2048, E=8 experts, I=5632, top_k=2, fp32.
Sharding: expert-parallel - core c owns expert c (w_gate/w_up/w_down[c]),
router weight replicated (rolled per-core so column 0 is the own expert).
Each core computes the full router (softmax + top-2 mask), its expert's
GLU-MLP for all 32 tokens, scales by its combine weight, and returns a
partial [T,H] output; the host sums the 8 partials.

The kernel is DMA-bound: per-core weight traffic dominates at
3*H*I bytes/elem. Weights and activations stream as fp16 (host-side
cast; end-to-end rel err ~5e-4 vs the 2e-2 gate), halving HBM traffic
vs fp32/f32r: 69.2 MB @ ~360 GB/s = ~193 us/core floor. Matmuls keep
the 32-token activations stationary ([128,32] lhsT tiles) and stream
the weights as the moving operand (1 cycle/row in fp16); PSUM
accumulates fp32. The combine weight is folded into the intermediate
activations before the down matmul, so the final output DMAs straight
out of the PSUM accumulators with no epilogue on the critical tail.

TRN2 allows only ONE sync-wait per instruction; the Bacc layer splits
excess waits into event-semaphore chains. The kernel is arranged so
most PE instructions need at most one new semaphore (operand pairs
arrive in a single DMA, junk transposes absorb DMA ticks), keeping the
inserted event chains off the hot path.
"""

import numpy as np

import concourse.bass as bass
import concourse.bacc as bacc
import concourse.mybir as mybir
import concourse.tile as tile
from concourse.masks import make_identity
from concourse.bass_utils import run_bass_kernel_spmd

B, S, H = 32, 1, 2048
E, I = 8, 5632
T = B * S          # 32 tokens
P = 128            # partitions
NCORES = 8
SWIGLU_SCALE = 1.702

KH = H // P        # 16 contraction chunks over H
KI = I // P        # 44 contraction chunks over I
NW = 512           # moving-dim tile width
NT = I // NW       # 11 gate/up column slabs
ND = H // NW       # 4 down output tiles
XW = T + E         # packed xT+router width (40)

F32 = mybir.dt.float32
F16 = mybir.dt.float16
AX = mybir.AxisListType.X
AF = mybir.ActivationFunctionType
OP = mybir.AluOpType


def _build_nc() -> bass.Bass:
    nc = bacc.Bacc()

    WDT = F16
    xrw_d = nc.declare_dram_parameter("xrw", [H, XW], WDT, isOutput=False)
    wg_d = nc.declare_dram_parameter("wg", [H, I], WDT, isOutput=False)
    wu_d = nc.declare_dram_parameter("wu", [H, I], WDT, isOutput=False)
    wd_d = nc.declare_dram_parameter("wd", [I, H], WDT, isOutput=False)
    out_d = nc.declare_dram_parameter("out", [T, H], F32, isOutput=True)

    with tile.TileContext(nc) as tc:
        with tc.tile_pool(name="const", bufs=1) as const:
            id_sb = const.tile([T, T], F32, name="id_sb")
            make_identity(nc, id_sb)

            xrw_sb = const.tile([P, KH * XW], WDT, name="xrw_sb")
            nc.sync.dma_start(
                out=xrw_sb.rearrange("p (k c) -> p k c", c=XW),
                in_=xrw_d.rearrange("(k p) c -> p k c", p=P),
            )

            interT_sb = const.tile([P, KI * T], WDT, name="interT_sb")
            comb_sb = const.tile([T, 1], F32, name="comb_sb")

            def xT_k(k):  # [128, 32] stationary activation chunk
                return xrw_sb[:, k * XW : k * XW + T]

            def rw_k(k):  # [128, 8] router weight chunk
                return xrw_sb[:, k * XW + T : (k + 1) * XW]

            widths = [NW] * (NT - 1) + [NW // 2, NW // 2]
            assert sum(widths) == I
            wg_cols = wg_d.rearrange("(k p) i -> p k i", p=P)
            wu_cols = wu_d.rearrange("(k p) i -> p k i", p=P)
            wgp = tc.alloc_tile_pool(name="wgp", bufs=2)
            wup = tc.alloc_tile_pool(name="wup", bufs=2)
            wdp = tc.alloc_tile_pool(name="wdp", bufs=3)

            # ---------------- router: softmax + top-2 mask ----------------
            with (
                tc.tile_pool(name="rps", bufs=1, space="PSUM") as rps,
                tc.tile_pool(name="rsb", bufs=1) as rsb,
            ):
                # absorb the ident DMA tick on PE before anything else
                dmy_ps = rps.tile([T, T], F32, name="dmy_ps", tag="dmy")
                nc.tensor.transpose(dmy_ps, id_sb, id_sb)

                logits = rps.tile([T, E], F32, name="logits", tag="logits")
                for k in range(KH):
                    nc.tensor.matmul(
                        logits,
                        xT_k(k),
                        rw_k(k),
                        start=(k == 0),
                        stop=(k == KH - 1),
                    )
                # PSUM is read only by DVE (keeps later PE writers 1-wait)
                lg = rsb.tile([T, E], F32, name="lg")
                nc.vector.tensor_copy(lg, logits)
                mx = rsb.tile([T, 1], F32, name="mx")
                nc.vector.reduce_max(mx, lg, axis=AX)
                nmx = rsb.tile([T, 1], F32, name="nmx")
                nc.vector.tensor_scalar_mul(nmx, mx, -1.0)
                ex = rsb.tile([T, E], F32, name="ex")
                nc.scalar.activation(ex, lg, AF.Exp, bias=nmx, scale=1.0)
                sm = rsb.tile([T, 1], F32, name="sm")
                nc.vector.reduce_sum(sm, ex, axis=AX)
                rc = rsb.tile([T, 1], F32, name="rc")
                nc.vector.reciprocal(rc, sm)
                aff = rsb.tile([T, E], F32, name="aff")
                nc.vector.tensor_scalar_mul(aff, ex, rc)
                # top-2: value >= (second largest)
                m1 = rsb.tile([T, 1], F32, name="m1")
                nc.vector.reduce_max(m1, aff, axis=AX)
                pen = rsb.tile([T, E], F32, name="pen")
                nc.vector.tensor_scalar(
                    pen, aff, m1, -1e30, op0=OP.is_equal, op1=OP.mult
                )
                b2 = rsb.tile([T, E], F32, name="b2")
                nc.vector.tensor_add(b2, aff, pen)
                m2 = rsb.tile([T, 1], F32, name="m2")
                nc.vector.reduce_max(m2, b2, axis=AX)
                ge = rsb.tile([T, E], F32, name="ge")
                nc.vector.tensor_scalar(ge, aff, m2, None, op0=OP.is_ge)
                msk = rsb.tile([T, E], F32, name="msk")
                nc.vector.tensor_mul(msk, aff, ge)
                # rolled router weight puts the own expert at column 0
                nc.vector.tensor_copy(comb_sb, msk[:, 0:1])

            # ---- fused gate/up + swiglu + transpose + interleaved down ----
            # Down-matmul chunk ki only needs interT chunk ki (ready right
            # after slab ki//4's epilogue), so wd streams through the whole
            # kernel instead of serializing after gate/up. Slab widths taper
            # to 256 at the end to shrink the serial tail chain after the
            # last weight bytes land. The combine weight is folded into
            # inter, so d_ps holds the final scaled output and DMAs out
            # directly. PSUM: gate/up 2 + transpose 2 + down accum 4 = 8.
            with (
                tc.tile_pool(name="gup", bufs=1, space="PSUM") as gup,
                tc.tile_pool(name="tps", bufs=2, space="PSUM") as tps,
                tc.tile_pool(name="dps", bufs=1, space="PSUM") as dps,
                tc.tile_pool(name="esb", bufs=2) as esb,
            ):
                d_ps = [
                    dps.tile([T, NW], F32, name=f"d_ps{j}", tag=f"d{j}")
                    for j in range(ND)
                ]
                c0 = 0
                for n, w in enumerate(widths):
                    wg_sl = wgp.tile([P, KH * NW], WDT, name="wg_sl", tag="wg")
                    wu_sl = wup.tile([P, KH * NW], WDT, name="wu_sl", tag="wu")
                    if w == NW:
                        nc.sync.dma_start(
                            out=wg_sl.rearrange("p (k c) -> p k c", c=NW)[:, :, :w],
                            in_=wg_cols[:, :, c0 : c0 + w],
                        )
                        nc.sync.dma_start(
                            out=wu_sl.rearrange("p (k c) -> p k c", c=NW)[:, :, :w],
                            in_=wu_cols[:, :, c0 : c0 + w],
                        )
                    else:
                        # tail slabs: k-halves so matmuls overlap the stream
                        for kh in (slice(0, KH // 2), slice(KH // 2, KH)):
                            nc.sync.dma_start(
                                out=wg_sl.rearrange("p (k c) -> p k c", c=NW)[
                                    :, kh, :w
                                ],
                                in_=wg_cols[:, kh, c0 : c0 + w],
                            )
                        for kh in (slice(0, KH // 2), slice(KH // 2, KH)):
                            nc.sync.dma_start(
                                out=wu_sl.rearrange("p (k c) -> p k c", c=NW)[
                                    :, kh, :w
                                ],
                                in_=wu_cols[:, kh, c0 : c0 + w],
                            )
                    g_ps = gup.tile([T, NW], F32, name="g_ps", tag="g")
                    u_ps = gup.tile([T, NW], F32, name="u_ps", tag="u")
                    for k in range(KH):
                        nc.tensor.matmul(
                            g_ps[:, :w],
                            xT_k(k),
                            wg_sl[:, k * NW : k * NW + w],
                            start=(k == 0),
                            stop=(k == KH - 1),
                        )
                    for k in range(KH):
                        nc.tensor.matmul(
                            u_ps[:, :w],
                            xT_k(k),
                            wu_sl[:, k * NW : k * NW + w],
                            start=(k == 0),
                            stop=(k == KH - 1),
                        )
                    # epilogue: PSUM read only by DVE; sigmoid runs off a copy
                    g_sb = esb.tile([T, NW], F32, name="g_sb", tag="gsb")
                    nc.vector.tensor_copy(g_sb[:, :w], g_ps[:, :w])
                    sig = esb.tile([T, NW], F32, name="sig", tag="sig")
                    nc.scalar.activation(
                        sig[:, :w], g_sb[:, :w], AF.Sigmoid, scale=SWIGLU_SCALE
                    )
                    t1 = esb.tile([T, NW], F32, name="t1", tag="t1")
                    nc.vector.tensor_mul(t1[:, :w], g_ps[:, :w], sig[:, :w])
                    t2 = esb.tile([T, NW], F32, name="t2", tag="t2")
                    nc.vector.tensor_mul(t2[:, :w], t1[:, :w], u_ps[:, :w])
                    # fold the combine weight in before the down matmul
                    inter = esb.tile([T, NW], F32, name="inter", tag="inter")
                    nc.vector.tensor_scalar_mul(inter[:, :w], t2[:, :w], comb_sb)
                    for j in range(w // P):
                        ic = c0 // P + j
                        tp = tps.tile([P, T], F32, name="tp", tag="tp")
                        nc.tensor.transpose(tp, inter[:, j * P : (j + 1) * P], id_sb)
                        nc.vector.tensor_copy(
                            interT_sb[:, ic * T : (ic + 1) * T], tp
                        )
                    for ki in range(c0 // P, (c0 + w) // P):
                        wd_sl = wdp.tile([P, H], WDT, name="wd_sl", tag="wd")
                        nc.gpsimd.dma_start(
                            out=wd_sl, in_=wd_d[ki * P : (ki + 1) * P, :]
                        )
                        for j in range(ND):
                            nc.tensor.matmul(
                                d_ps[j],
                                interT_sb[:, ki * T : (ki + 1) * T],
                                wd_sl[:, j * NW : (j + 1) * NW],
                                start=(ki == 0),
                                stop=(ki == KI - 1),
                            )
                    c0 += w
                out_sb = const.tile([T, H], F32, name="out_sb")
                for j in range(ND):
                    nc.vector.tensor_copy(
                        out_sb[:, j * NW : (j + 1) * NW], d_ps[j]
                    )
                    nc.sync.dma_start(
                        out=out_d[:, j * NW : (j + 1) * NW],
                        in_=out_sb[:, j * NW : (j + 1) * NW],
                    )
            wdp.release()
            wup.release()
            wgp.release()
    nc.finalize()
    return nc


def _make_in_maps(hidden_states, router_weight, w_gate, w_up, w_down):
    x = np.asarray(hidden_states, np.float32).reshape(T, H)
    rw = np.asarray(router_weight, np.float32)
    wg = np.ascontiguousarray(np.asarray(w_gate, np.float16))
    wu = np.ascontiguousarray(np.asarray(w_up, np.float16))
    wd = np.ascontiguousarray(np.asarray(w_down, np.float16))
    xT = np.ascontiguousarray(x.T.astype(np.float16))  # [H, T]
    in_maps = []
    for c in range(NCORES):
        order = [(j + c) % E for j in range(E)]  # column j holds expert (j+c)%E
        rwT = rw[order].T.astype(np.float16)  # [H, E]; col 0 = own expert
        xrw = np.ascontiguousarray(np.concatenate([xT, rwT], axis=1))  # [H, T+E]
        in_maps.append(
            {
                "xrw": xrw,
                "wg": wg[c],
                "wu": wu[c],
                "wd": wd[c],
            }
        )
    return in_maps


def kernel(
    hidden_states,
    router_weight,
    w_gate,
    w_up,
    w_down,
    top_k,
    _trace: bool = False,
    _trace_all: bool = False,
    **_unused,
):
    assert int(top_k) == 2, "kernel hardcodes top_k=2"
    in_maps = _make_in_maps(hidden_states, router_weight, w_gate, w_up, w_down)
    nc = _build_nc()
    res = run_bass_kernel_spmd(
        nc, in_maps, core_ids=list(range(NCORES)), trace=_trace,
        trace_cores=list(range(NCORES)) if (_trace and _trace_all) else None,
    )
    outs = np.stack([res.results[c]["out"] for c in range(NCORES)], axis=0)
    out = outs.sum(axis=0, dtype=np.float64).astype(np.float32)
    if _trace:
        kernel.last_exec_time_ns = res.exec_time_ns
        kernel.last_mean_exec_time_ns = res.mean_exec_time_ns
        kernel.last_trace = res.instructions_and_trace
    return out.reshape(B, S, H)
